# revision 1
# baseline (speedup 1.0000x reference)
"""Trainium2 Bass kernel for nn_DecoderLSTM (N=262144, H=64, IN=66, PRED_LEN=12).

Strategy (pure data parallel over 8 NeuronCores, N/8 = 32768 rows per core):

  * All state is kept TRANSPOSED on chip: [feature-dim on partitions,
    batch on free dim].  The recurrent matmuls then need no per-step
    transposes: gates_T[gate, b] = W @ x_T, computed as PE matmuls with
    the small weights as the stationary operand (lhsT) and the batch
    streaming (rhs).
  * Batch is processed in pairs of 512-element chunks: chunk X lives on
    partitions 0..63, chunk Y on partitions 64..127 of every tile, so
    every DVE/ACT instruction runs at the full 128-partition width.
    The per-gate weights are embedded as 128x128 block-diagonal (or
    block-structured) matrices so one matmul produces one gate for both
    chunks.
  * Three pairs are software-pipelined (their steps interleaved in
    program order) so PE / ACT / DVE work of different pairs overlaps;
    PSUM: one [128,1536] I|F|O tile (bufs=2) + one [128,512] tile shared
    by G and the fc delta (bufs=2) = exactly 8 banks.
  * The constant-over-time context contribution is re-streamed into the
    PSUM accumulation each step.  The position (2 dims) + gate bias ride
    a K=3 matmul whose stationary block contains [Wp.T; bias-row]; rows
    32/96 of the pos tile are pinned to 1.0 so the bias comes for free.
  * pos_{t+1} = pos_t + (h_{t+1} @ fc_w.T) + fc_b is one
    scalar_tensor_tensor on DVE; each step's positions are DMAd straight
    to DRAM.  The pos/delta partition rows rotate over 8 variants
    (rows {4k,4k+1} / {64+4k,64+4k+1}) so output DMAs spread across all
    16 SDMA engines instead of hammering partitions 0/1/64/65.
  * Matmuls run in float32r (full-rate fp32 PE mode); PSUM accumulates
    in fp32; all elementwise math in fp32.

Device output layout is [12, 2, NC] per core (batch-contiguous);
the host glues the 8 shards and transposes to [N, 12, 2].
"""

import numpy as np

import concourse.bass as bass
import concourse.bacc as bacc_mod
import concourse.mybir as mybir
import concourse.tile as tile
from concourse.bass import ds, ts
from concourse.bass_utils import run_bass_kernel_spmd

N_CORES = 8
N_TOTAL = 262144
NCB = N_TOTAL // N_CORES  # 32768 batch rows per core
H = 64
PRED = 12
NB = 512                 # batch elements per chunk (one PSUM bank @ fp32)
PAIRS = NCB // (2 * NB)  # 32 chunk-pairs per core
NVAR = 8                 # pos/delta partition-row variants
INTERLEAVE = 3           # pairs software-pipelined together

F32 = mybir.dt.float32
BF16 = mybir.dt.bfloat16
AF = mybir.ActivationFunctionType
ALU = mybir.AluOpType

# gates tile bank order: I, F, O (sigmoid, one merged ACT op) ; G separate
_BANK_GATE = ("i", "f", "o", "g")
_GATE_SLICE = {"i": slice(0, 64), "f": slice(64, 128),
               "g": slice(128, 192), "o": slice(192, 256)}

# weight-block layout (128 cols each) in the packed [128, NWB*128] tensor:
#   0..3           W_HH   (I, F, O, G)
#   4..7           W_CTX  (I, F, O, G)
#   8..8+4*NVAR-1  W_POS  variant-major: v*4 + bank
#   8+4*NVAR..+NVAR-1   W_FC variants
#   last           FCBS: col v holds fc_b at the variant's 4 rows
_POS0 = 8
_FC0 = _POS0 + 4 * NVAR
_FCB0 = _FC0 + NVAR
N_WBLK = _FCB0 + 1


def _var_rows(v):
    return 4 * v, 64 + 4 * v  # X row base, Y row base


def build_weight_blocks(W_ih, W_hh, b_ih, b_hh, fc_w, fc_b):
    b = (b_ih + b_hh).astype(np.float32)
    out = np.zeros((128, N_WBLK * 128), dtype=np.float32)

    def blk(j):
        return out[:, 128 * j:128 * (j + 1)]

    for j, gate in enumerate(_BANK_GATE):
        sl = _GATE_SLICE[gate]
        whh_t = W_hh[sl, :].T.astype(np.float32)          # [64(h), 64(out)]
        wc_t = W_ih[sl, 2:66].T.astype(np.float32)        # [64(ctx), 64(out)]
        wp_t = W_ih[sl, 0:2].T.astype(np.float32)         # [2, 64(out)]
        bg = b[sl]

        blk(j)[0:64, 0:64] = whh_t
        blk(j)[64:128, 64:128] = whh_t
        blk(4 + j)[0:64, 0:64] = wc_t
        blk(4 + j)[64:128, 64:128] = wc_t

        for v in range(NVAR):
            rx, ry = _var_rows(v)
            pb = blk(_POS0 + 4 * v + j)
            pb[rx:rx + 2, 0:64] = wp_t
            pb[32, 0:64] = bg
            pb[ry:ry + 2, 64:128] = wp_t
            pb[96, 64:128] = bg

    fcw_t = fc_w.T.astype(np.float32)                     # [64, 2]
    fcbv = np.zeros((128, NVAR), dtype=np.float32)
    for v in range(NVAR):
        rx, ry = _var_rows(v)
        fb = blk(_FC0 + v)
        fb[0:64, rx:rx + 2] = fcw_t
        fb[64:128, ry:ry + 2] = fcw_t
        fcbv[rx:rx + 2, v] = fc_b.astype(np.float32)
        fcbv[ry:ry + 2, v] = fc_b.astype(np.float32)
    return out, fcbv


class _PairState:
    __slots__ = ("p", "v", "xb", "yb", "ctx", "pos", "posb", "h", "c")


def build_bass(pairs=PAIRS):
    """Trace the per-core Tile kernel (identical on all 8 cores)."""
    nc = bacc_mod.Bacc()
    ctxT = nc.declare_dram_parameter("ctxT", [H, NCB], BF16, isOutput=False)
    posT = nc.declare_dram_parameter("posT", [2, NCB], F32, isOutput=False)
    wblk = nc.declare_dram_parameter("wblk", [128, N_WBLK * 128], BF16,
                                     isOutput=False)
    fcbv = nc.declare_dram_parameter("fcbv", [128, NVAR], F32, isOutput=False)
    out = nc.declare_dram_parameter("out", [PRED, 2, NCB], F32, isOutput=True)

    with tile.TileContext(nc) as tc:
        with (
            tc.tile_pool(name="wpool", bufs=1) as wpool,
            tc.tile_pool(name="sb", bufs=2) as sb,
            tc.tile_pool(name="psum", bufs=2, space="PSUM") as psum,
        ):
            wt = wpool.tile([128, N_WBLK * 128], BF16, name="wt")
            nc.sync.dma_start(out=wt[:, :], in_=wblk[:, :])
            fcbt = wpool.tile([128, NVAR], F32, name="fcbt")
            nc.sync.dma_start(out=fcbt[:, :], in_=fcbv[:, :])
            WHH = [wt[:, ts(j, 128)] for j in range(4)]
            WCTX = [wt[:, ts(4 + j, 128)] for j in range(4)]

            def wpos(v, bk):
                return wt[:, ts(_POS0 + 4 * v + bk, 128)]

            def wfc(v):
                return wt[:, ts(_FC0 + v, 128)]

            def fcb(v):
                return fcbt[:, v:v + 1]

            def prologue(st):
                st.ctx = sb.tile([128, NB], BF16, tag=f"ctx{st.p % INTERLEAVE}")
                nc.sync.dma_start(out=st.ctx[0:64, :],
                                  in_=ctxT[:, ds(st.xb, NB)])
                nc.sync.dma_start(out=st.ctx[64:128, :],
                                  in_=ctxT[:, ds(st.yb, NB)])
                rx, ry = _var_rows(st.v)
                j = st.p % INTERLEAVE
                st.pos = sb.tile([128, NB], F32, tag=f"pos{j}")
                nc.vector.memset(st.pos[:, :], 0.0)
                nc.sync.dma_start(out=st.pos[rx:rx + 2, :],
                                  in_=posT[:, ds(st.xb, NB)])
                nc.sync.dma_start(out=st.pos[ry:ry + 2, :],
                                  in_=posT[:, ds(st.yb, NB)])
                nc.vector.memset(st.pos[32:33, :], 1.0)
                nc.vector.memset(st.pos[96:97, :], 1.0)
                st.posb = sb.tile([128, NB], BF16, tag=f"posb{j}")
                nc.vector.tensor_copy(st.posb[:, :], st.pos[:, :])
                st.h = None
                st.c = None

            def step(st, t):
                j = st.p % INTERLEAVE
                rx, ry = _var_rows(st.v)
                g1 = psum.tile([128, 3 * NB], F32, tag="g1")
                g2 = psum.tile([128, NB], F32, tag="g2")
                ctx_rhs = st.ctx[:, :]
                pos_rhs = st.posb[:, :]
                for bk in range(4):
                    bank = g1[:, ts(bk, NB)] if bk < 3 else g2[:, :]
                    nc.tensor.matmul(bank, WCTX[bk], ctx_rhs,
                                     start=True, stop=False)
                    nc.tensor.matmul(bank, wpos(st.v, bk), pos_rhs,
                                     start=False, stop=(t == 0))
                    if t > 0:
                        nc.tensor.matmul(bank, WHH[bk], st.h[:, :],
                                         start=False, stop=True)

                sb_ifo = sb.tile([128, 3 * NB], BF16, tag=f"ifo{j}")
                sb_g = sb.tile([128, NB], BF16, tag=f"g{j}")
                nc.scalar.activation(sb_ifo[:, :], g1[:, :], AF.Sigmoid)
                nc.scalar.activation(sb_g[:, :], g2[:, :], AF.Tanh)

                c_new = sb.tile([128, NB], F32, tag=f"c{j}")
                if t == 0:
                    nc.vector.tensor_mul(c_new[:, :], sb_ifo[:, 0:NB],
                                         sb_g[:, :])
                else:
                    p1 = sb.tile([128, NB], BF16, tag=f"p1{j}", bufs=1)
                    p2 = sb.tile([128, NB], F32, tag=f"p2{j}", bufs=1)
                    nc.vector.tensor_mul(p1[:, :], sb_ifo[:, 0:NB], sb_g[:, :])
                    nc.vector.tensor_mul(p2[:, :], sb_ifo[:, ds(NB, NB)],
                                         st.c[:, :])
                    nc.vector.tensor_add(c_new[:, :], p1[:, :], p2[:, :])

                sb_tc = sb.tile([128, NB], BF16, tag=f"tc{j}", bufs=1)
                nc.scalar.activation(sb_tc[:, :], c_new[:, :], AF.Tanh)

                h_new = sb.tile([128, NB], BF16, tag=f"h{j}")
                nc.vector.tensor_mul(h_new[:, :], sb_ifo[:, ds(2 * NB, NB)],
                                     sb_tc[:, :])

                delta = psum.tile([128, NB], F32, tag="g2")
                nc.tensor.matmul(delta[:, :], wfc(st.v), h_new[:, :],
                                 start=True, stop=True)

                pos_new = sb.tile([128, NB], F32, tag=f"pos{j}")
                nc.vector.scalar_tensor_tensor(
                    out=pos_new[:, :], in0=delta[:, :], scalar=fcb(st.v),
                    in1=st.pos[:, :], op0=ALU.add, op1=ALU.add)
                if t + 1 < PRED:
                    posb_new = sb.tile([128, NB], BF16, tag=f"posb{j}")
                    nc.vector.tensor_copy(posb_new[:, :], pos_new[:, :])
                    st.posb = posb_new

                nc.sync.dma_start(out=out[t, :, ds(st.xb, NB)],
                                  in_=pos_new[rx:rx + 2, :])
                nc.sync.dma_start(out=out[t, :, ds(st.yb, NB)],
                                  in_=pos_new[ry:ry + 2, :])

                st.pos = pos_new
                st.h = h_new
                st.c = c_new

            done = 0
            while done < pairs:
                grp = min(INTERLEAVE, pairs - done)
                sts = []
                for k in range(grp):
                    st = _PairState()
                    st.p = done + k
                    st.v = st.p % NVAR
                    st.xb = (2 * st.p) * NB
                    st.yb = (2 * st.p + 1) * NB
                    prologue(st)
                    sts.append(st)
                for t in range(PRED):
                    for st in sts:
                        step(st, t)
                done += grp
    nc.finalize()
    return nc


def make_in_maps(context, start_pos, W_ih, W_hh, b_ih, b_hh, fc_w, fc_b):
    context = np.asarray(context, dtype=np.float32)
    start_pos = np.asarray(start_pos, dtype=np.float32)
    import ml_dtypes
    ctxT = np.ascontiguousarray(context.T.astype(ml_dtypes.bfloat16))  # [64, N]
    posT = np.ascontiguousarray(start_pos.T)    # [2, N]
    wb, fcbv = build_weight_blocks(
        np.asarray(W_ih, np.float32), np.asarray(W_hh, np.float32),
        np.asarray(b_ih, np.float32), np.asarray(b_hh, np.float32),
        np.asarray(fc_w, np.float32), np.asarray(fc_b, np.float32))
    wb = wb.astype(ml_dtypes.bfloat16)
    in_maps = []
    for c in range(N_CORES):
        sl = slice(c * NCB, (c + 1) * NCB)
        in_maps.append({
            "ctxT": np.ascontiguousarray(ctxT[:, sl]),
            "posT": np.ascontiguousarray(posT[:, sl]),
            "wblk": wb,
            "fcbv": fcbv,
        })
    return in_maps


def gather_output(results):
    outs = [np.asarray(results[c]["out"]) for c in range(N_CORES)]
    full = np.concatenate(outs, axis=2)          # [12, 2, N]
    return np.ascontiguousarray(np.transpose(full, (2, 0, 1)).astype(np.float32))


def kernel(context, start_pos, W_ih, W_hh, b_ih, b_hh, fc_w, fc_b):
    in_maps = make_in_maps(context, start_pos, W_ih, W_hh, b_ih, b_hh,
                           fc_w, fc_b)
    nc = build_bass()
    res = run_bass_kernel_spmd(nc, in_maps, list(range(N_CORES)))
    return gather_output(res.results)



# revision 22
# speedup vs baseline: 1204.4740x; 1204.4740x over previous
"""Trainium2 Bass kernel for nn_DecoderLSTM (N=262144, H=64, IN=66, PRED_LEN=12).

Strategy (pure data parallel over 8 NeuronCores, N/8 = 32768 rows per core):

  * All state is kept TRANSPOSED on chip: [feature-dim on partitions,
    batch on free dim].  The recurrent matmuls then need no per-step
    transposes: gates_T[gate, b] = W @ x_T, computed as PE matmuls with
    the small weights as the stationary operand (lhsT) and the batch
    streaming (rhs).
  * Batch is processed in pairs of 512-element chunks: chunk X lives on
    partitions 0..63, chunk Y on partitions 64..127 of every tile, so
    every DVE/ACT instruction runs at the full 128-partition width.
    The per-gate weights are embedded as 128x128 block-diagonal (or
    block-structured) matrices so one matmul produces one gate for both
    chunks.
  * Four pairs are software-pipelined.  The pos feedback is decoupled
    from the critical path by folding the fc head into the recurrent
    weights (Whh' = Whh.T + fcw.T @ Wp.T, bias += fcb @ Wp.T, and step 0
    streams pos_0 - fcb), so gates at step t need only pos_{t-1}: the
    fc matmul / pos update / bf16 copy all run one step behind with a
    full step of slack, and no engine's in-order queue blocks while
    another pair has ready work.  tanh(c) is emitted inline per pair so
    it fills the ACT engine's wait-for-matmul gaps.
  * PSUM: one [128,1536] I|O|F tile (bufs=2) + one [128,512] tile shared
    by G and the fc delta (bufs=2) = exactly 8 banks.  Bank order I|O|F
    lets step 0 skip the F gate entirely (it multiplies c_0 == 0).
  * The constant-over-time context contribution is re-streamed into the
    PSUM accumulation each step.  The position (2 dims) + gate bias ride
    a K=3 matmul whose stationary block contains [Wp.T; bias-row]; rows
    32/96 of the pos tile are pinned to 1.0 so the bias comes for free.
  * pos_{t+1} = pos_t + (h_{t+1} @ fc_w.T) + fc_b is one
    scalar_tensor_tensor on DVE; each step's positions are DMAd straight
    to DRAM.  X/Y chunk pos rows are adjacent ({4v..4v+3}, rotating over
    8 variants to spread DMA queues), so each step's output is a single
    4-partition DMA; ctx/pos input DMAs are likewise merged via
    rearranged access patterns.
  * The cell state c is kept in bf16 (tolerance is 2e-2) so every
    c/h elementwise op runs in the DVE's 2x packed mode.

Device output layout is [12, 2, NC] per core (batch-contiguous);
the host glues the 8 shards and transposes to [N, 12, 2].
"""

import numpy as np

import concourse.bass as bass
import concourse.bacc as bacc_mod
import concourse.mybir as mybir
import concourse.tile as tile
from concourse.bass import ds, ts
from concourse.bass_utils import run_bass_kernel_spmd

N_CORES = 8
N_TOTAL = 262144
NCB = N_TOTAL // N_CORES  # 32768 batch rows per core
H = 64
PRED = 12
NB = 512                 # batch elements per chunk (one PSUM bank @ fp32)
PAIRS = NCB // (2 * NB)  # 32 chunk-pairs per core
NVAR = 8                 # pos/delta partition-row variants
INTERLEAVE = 4           # pairs software-pipelined together

F32 = mybir.dt.float32
BF16 = mybir.dt.bfloat16
AF = mybir.ActivationFunctionType
ALU = mybir.AluOpType

# gates tile bank order: I, F, O (sigmoid, one merged ACT op) ; G separate
_BANK_GATE = ("i", "o", "f", "g")
_GATE_SLICE = {"i": slice(0, 64), "f": slice(64, 128),
               "g": slice(128, 192), "o": slice(192, 256)}

# weight-block layout (128 cols each) in the packed [128, NWB*128] tensor:
#   0..3           W_HH   (I, F, O, G)
#   4..7           W_CTX  (I, F, O, G)
#   8..8+4*NVAR-1  W_POS  variant-major: v*4 + bank
#   8+4*NVAR..+NVAR-1   W_FC variants
#   last           FCBS: col v holds fc_b at the variant's 4 rows
_POS0 = 8
_FC0 = _POS0 + 4 * NVAR
_FCB0 = _FC0 + NVAR
N_WBLK = _FCB0 + 1


def _var_rows(v):
    # X pos at rows {4v, 4v+1}, Y pos adjacent at {4v+2, 4v+3}: the output
    # (and input) pos DMAs then move 4 contiguous partitions in one shot
    return 4 * v, 4 * v + 2


def build_weight_blocks(W_ih, W_hh, b_ih, b_hh, fc_w, fc_b):
    b = (b_ih + b_hh).astype(np.float32)
    out = np.zeros((128, N_WBLK * 128), dtype=np.float32)

    def blk(j):
        return out[:, 128 * j:128 * (j + 1)]

    fcw_t = fc_w.T.astype(np.float32)                     # [64, 2]
    for j, gate in enumerate(_BANK_GATE):
        sl = _GATE_SLICE[gate]
        wp_t = W_ih[sl, 0:2].T.astype(np.float32)         # [2, 64(out)]
        # pos-chain decoupling: substitute pos_t = pos_{t-1} + h_t@fcw.T+fcb
        # into the gate equation, so gates at step t depend on pos_{t-1}
        # (one extra step of pipeline slack).  h carries the correction via
        # Whh' = Whh.T + fcw.T @ Wp.T and the bias row gains fcb @ Wp.T;
        # step 0 is fixed up by streaming (pos_0 - fcb) instead of pos_0.
        whh_t = W_hh[sl, :].T.astype(np.float32) + fcw_t @ wp_t
        wc_t = W_ih[sl, 2:66].T.astype(np.float32)        # [64(ctx), 64(out)]
        bg = b[sl] + fc_b.astype(np.float32) @ wp_t

        blk(j)[0:64, 0:64] = whh_t
        blk(j)[64:128, 64:128] = whh_t
        blk(4 + j)[0:64, 0:64] = wc_t
        blk(4 + j)[64:128, 64:128] = wc_t

        for v in range(NVAR):
            rx, ry = _var_rows(v)
            pb = blk(_POS0 + 4 * v + j)
            pb[rx:rx + 2, 0:64] = wp_t
            pb[32, 0:64] = bg
            pb[ry:ry + 2, 64:128] = wp_t
            pb[96, 64:128] = bg

    fcbv = np.zeros((128, NVAR), dtype=np.float32)
    for v in range(NVAR):
        rx, ry = _var_rows(v)
        fb = blk(_FC0 + v)
        fb[0:64, rx:rx + 2] = fcw_t
        fb[64:128, ry:ry + 2] = fcw_t
        fcbv[rx:rx + 2, v] = fc_b.astype(np.float32)
        fcbv[ry:ry + 2, v] = fc_b.astype(np.float32)
    return out, fcbv


class _PairState:
    __slots__ = ("p", "v", "xb", "yb", "ctx", "pos", "posb", "h", "c",
                 "posb_nxt", "ifo", "g", "p1", "p2", "tc", "delta")


def build_bass(pairs=PAIRS, repeat=1):
    """Trace the per-core Tile kernel (identical on all 8 cores)."""
    nc = bacc_mod.Bacc()
    ctxT = nc.declare_dram_parameter("ctxT", [H, NCB], BF16, isOutput=False)
    posT = nc.declare_dram_parameter("posT", [2, NCB], F32, isOutput=False)
    wblk = nc.declare_dram_parameter("wblk", [128, N_WBLK * 128], BF16,
                                     isOutput=False)
    fcbv = nc.declare_dram_parameter("fcbv", [128, NVAR], F32, isOutput=False)
    out = nc.declare_dram_parameter("out", [PRED, 2, NCB], F32, isOutput=True)

    with tile.TileContext(nc) as tc:
        with (
            tc.tile_pool(name="wpool", bufs=1) as wpool,
            tc.tile_pool(name="sb", bufs=2) as sb,
            tc.tile_pool(name="psum", bufs=2, space="PSUM") as psum,
        ):
            wt = wpool.tile([128, N_WBLK * 128], BF16, name="wt")
            nc.sync.dma_start(out=wt[:, :], in_=wblk[:, :])
            fcbt = wpool.tile([128, NVAR], F32, name="fcbt")
            nc.sync.dma_start(out=fcbt[:, :], in_=fcbv[:, :])
            WHH = [wt[:, ts(j, 128)] for j in range(4)]
            WCTX = [wt[:, ts(4 + j, 128)] for j in range(4)]

            def wpos(v, bk):
                return wt[:, ts(_POS0 + 4 * v + bk, 128)]

            def wfc(v):
                return wt[:, ts(_FC0 + v, 128)]

            def fcb(v):
                return fcbt[:, v:v + 1]

            def prologue(st):
                j = st.p % INTERLEAVE
                st.ctx = sb.tile([128, NB], BF16, tag=f"ctx{j}")
                nc.sync.dma_start(
                    out=st.ctx[:, :],
                    in_=ctxT[:, ds(st.xb, 2 * NB)].rearrange(
                        "h (c b) -> c h b", c=2))
                rx, ry = _var_rows(st.v)
                st.pos = sb.tile([128, NB], F32, tag=f"pos{j}", bufs=3)
                nc.vector.memset(st.pos[:, :], 0.0)
                nc.vector.memset(st.pos[32:33, :], 1.0)
                nc.vector.memset(st.pos[96:97, :], 1.0)
                nc.sync.dma_start(
                    out=st.pos[ds(rx, 4), :],
                    in_=posT[:, ds(st.xb, 2 * NB)].rearrange(
                        "p (c b) -> c p b", c=2))
                # step 0 streams (pos_0 - fcb) through the cf-adjusted bias
                # row, which cancels back to pos_0 @ Wp.T + b exactly
                st.posb = sb.tile([128, NB], BF16, tag=f"posb{j}")
                nc.vector.tensor_scalar_sub(st.posb[:, :], st.pos[:, :],
                                            fcb(st.v))
                st.posb_nxt = sb.tile([128, NB], BF16, tag=f"posb{j}")
                nc.vector.tensor_copy(st.posb_nxt[:, :], st.pos[:, :])
                st.h = None
                st.c = None

            def emit_gates(st, t):
                """12 gate matmuls + the sigmoid/tanh reads of the banks."""
                j = st.p % INTERLEAVE
                g1 = psum.tile([128, 3 * NB], F32, tag="g1")
                g2 = psum.tile([128, NB], F32, tag="g2")
                for bk in range(4):
                    if t == 0 and bk == 2:
                        continue  # f gate multiplies c0 == 0
                    bank = g1[:, ts(bk, NB)] if bk < 3 else g2[:, :]
                    nc.tensor.matmul(bank, WCTX[bk], st.ctx[:, :],
                                     start=True, stop=False)
                    nc.tensor.matmul(bank, wpos(st.v, bk), st.posb[:, :],
                                     start=False, stop=(t == 0))
                    if t > 0:
                        nc.tensor.matmul(bank, WHH[bk], st.h[:, :],
                                         start=False, stop=True)
                st.posb = st.posb_nxt
                st.ifo = sb.tile([128, 3 * NB], BF16, tag=f"ifo{j}")
                st.g = sb.tile([128, NB], BF16, tag=f"g{j}")
                nio = 2 * NB if t == 0 else 3 * NB
                nc.scalar.activation(st.ifo[:, 0:nio], g1[:, 0:nio],
                                     AF.Sigmoid)
                nc.scalar.activation(st.g[:, :], g2[:, :], AF.Tanh)

            def emit_cupd(st, t):
                """c_new = f*c + i*g on DVE (bf16 2x mode)."""
                j = st.p % INTERLEAVE
                c_new = sb.tile([128, NB], BF16, tag=f"c{j}")
                if t == 0:
                    nc.vector.tensor_mul(c_new[:, :], st.ifo[:, 0:NB],
                                         st.g[:, :])
                else:
                    st.p1 = sb.tile([128, NB], BF16, tag=f"p1{j}", bufs=1)
                    st.p2 = sb.tile([128, NB], BF16, tag=f"p2{j}", bufs=1)
                    nc.vector.tensor_mul(st.p1[:, :], st.ifo[:, 0:NB],
                                         st.g[:, :])
                    nc.vector.tensor_mul(st.p2[:, :], st.ifo[:, ds(2 * NB, NB)],
                                         st.c[:, :])
                    nc.vector.tensor_add(c_new[:, :], st.p1[:, :],
                                         st.p2[:, :])
                st.c = c_new

            def emit_tanh_c(st, t):
                j = st.p % INTERLEAVE
                st.tc = sb.tile([128, NB], BF16, tag=f"tc{j}", bufs=1)
                nc.scalar.activation(st.tc[:, :], st.c[:, :], AF.Tanh)

            def emit_h(st, t):
                j = st.p % INTERLEAVE
                h_new = sb.tile([128, NB], BF16, tag=f"h{j}")
                nc.vector.tensor_mul(h_new[:, :], st.ifo[:, ds(NB, NB)],
                                     st.tc[:, :])
                st.h = h_new

            def emit_fc(st, t):
                st.delta = psum.tile([128, NB], F32, tag="g2")
                nc.tensor.matmul(st.delta[:, :], wfc(st.v), st.h[:, :],
                                 start=True, stop=True)

            def emit_pos(st, t):
                j = st.p % INTERLEAVE
                rx, ry = _var_rows(st.v)
                pos_new = sb.tile([128, NB], F32, tag=f"pos{j}", bufs=3)
                nc.vector.scalar_tensor_tensor(
                    out=pos_new[:, :], in0=st.delta[:, :], scalar=fcb(st.v),
                    in1=st.pos[:, :], op0=ALU.add, op1=ALU.add)
                if t + 2 < PRED:
                    posb_new = sb.tile([128, NB], BF16, tag=f"posb{j}")
                    nc.vector.tensor_copy(posb_new[:, :], pos_new[:, :])
                    st.posb_nxt = posb_new
                nc.sync.dma_start(
                    out=out[t, :, ds(st.xb, 2 * NB)].rearrange(
                        "p (c b) -> c p b", c=2),
                    in_=pos_new[ds(rx, 4), :])
                st.pos = pos_new

            def make_group(done, grp):
                sts = []
                for k in range(grp):
                    st = _PairState()
                    st.p = done + k
                    st.v = st.p % NVAR
                    st.xb = (2 * st.p) * NB
                    st.yb = (2 * st.p + 1) * NB
                    prologue(st)
                    sts.append(st)
                return sts

            for rep in range(repeat):
                done = 0
                sts = make_group(0, min(INTERLEAVE, pairs))
                while sts:
                    grp = len(sts)
                    done += grp
                    nxt = None
                    for t in range(PRED):
                        if t > 0:
                            # delta shares the g2 PSUM tag (bufs=2): fuse
                            # fc+pos per pair so each delta's stt read is
                            # emitted before the 3rd pair recycles its buffer
                            for st in sts:
                                emit_fc(st, t - 1)
                                emit_pos(st, t - 1)
                        if t == PRED - 1 and done < pairs:
                            # rolling prologue: next group's input DMAs and
                            # memsets overlap this group's tail latency chain
                            nxt = make_group(done, min(INTERLEAVE,
                                                       pairs - done))
                        for st in sts:
                            # inline per pair: tanh(c) lands between this
                            # pair's tg and the next pair's sigmoid, filling
                            # the ACT engine's wait-for-matmul gaps
                            emit_gates(st, t)
                            emit_cupd(st, t)
                            emit_tanh_c(st, t)
                            emit_h(st, t)
                    for st in sts:
                        emit_fc(st, PRED - 1)
                        emit_pos(st, PRED - 1)
                    sts = nxt if nxt is not None else []
    nc.finalize()
    return nc


def make_in_maps(context, start_pos, W_ih, W_hh, b_ih, b_hh, fc_w, fc_b):
    context = np.asarray(context, dtype=np.float32)
    start_pos = np.asarray(start_pos, dtype=np.float32)
    import ml_dtypes
    ctxT = np.ascontiguousarray(context.T.astype(ml_dtypes.bfloat16))  # [64, N]
    posT = np.ascontiguousarray(start_pos.T)    # [2, N]
    wb, fcbv = build_weight_blocks(
        np.asarray(W_ih, np.float32), np.asarray(W_hh, np.float32),
        np.asarray(b_ih, np.float32), np.asarray(b_hh, np.float32),
        np.asarray(fc_w, np.float32), np.asarray(fc_b, np.float32))
    wb = wb.astype(ml_dtypes.bfloat16)
    in_maps = []
    for c in range(N_CORES):
        sl = slice(c * NCB, (c + 1) * NCB)
        in_maps.append({
            "ctxT": np.ascontiguousarray(ctxT[:, sl]),
            "posT": np.ascontiguousarray(posT[:, sl]),
            "wblk": wb,
            "fcbv": fcbv,
        })
    return in_maps


def gather_output(results):
    outs = [np.asarray(results[c]["out"]) for c in range(N_CORES)]
    full = np.concatenate(outs, axis=2)          # [12, 2, N]
    return np.ascontiguousarray(np.transpose(full, (2, 0, 1)).astype(np.float32))


def kernel(context, start_pos, W_ih, W_hh, b_ih, b_hh, fc_w, fc_b):
    in_maps = make_in_maps(context, start_pos, W_ih, W_hh, b_ih, b_hh,
                           fc_w, fc_b)
    nc = build_bass()
    res = run_bass_kernel_spmd(nc, in_maps, list(range(N_CORES)))
    return gather_output(res.results)


# revision 25
# speedup vs baseline: 1249.5177x; 1.0374x over previous
"""Trainium2 Bass kernel for nn_DecoderLSTM (N=262144, H=64, IN=66, PRED_LEN=12).

Strategy (pure data parallel over 8 NeuronCores, N/8 = 32768 rows per core):

  * All state is kept TRANSPOSED on chip: [feature-dim on partitions,
    batch on free dim].  The recurrent matmuls then need no per-step
    transposes: gates_T[gate, b] = W @ x_T, computed as PE matmuls with
    the small weights as the stationary operand (lhsT) and the batch
    streaming (rhs).
  * Batch is processed in pairs of 512-element chunks: chunk X lives on
    partitions 0..63, chunk Y on partitions 64..127 of every tile, so
    every DVE/ACT instruction runs at the full 128-partition width.
    The per-gate weights are embedded as 128x128 block-diagonal (or
    block-structured) matrices so one matmul produces one gate for both
    chunks.
  * Four pairs are software-pipelined.  The pos feedback is decoupled
    from the critical path by folding the fc head into the recurrent
    weights (Whh' = Whh.T + fcw.T @ Wp.T, bias += fcb @ Wp.T, and step 0
    streams pos_0 - fcb), so gates at step t need only pos_{t-1}: the
    fc matmul / pos update / bf16 copy all run one step behind with a
    full step of slack, and no engine's in-order queue blocks while
    another pair has ready work.  tanh(c) is emitted inline per pair so
    it fills the ACT engine's wait-for-matmul gaps.
  * PSUM: one [128,1536] I|O|F tile (bufs=2) + one [128,512] tile shared
    by G and the fc delta (bufs=2) = exactly 8 banks.  Bank order I|O|F
    lets step 0 skip the F gate entirely (it multiplies c_0 == 0).
  * The constant-over-time context contribution is re-streamed into the
    PSUM accumulation each step.  The position (2 dims) + gate bias ride
    a K=3 matmul whose stationary block contains [Wp.T; bias-row]; rows
    32/96 of the pos tile are pinned to 1.0 so the bias comes for free.
  * pos_{t+1} = pos_t + (h_{t+1} @ fc_w.T) + fc_b is one
    scalar_tensor_tensor on DVE; each step's positions are DMAd straight
    to DRAM.  X/Y chunk pos rows are adjacent ({4v..4v+3}, rotating over
    8 variants to spread DMA queues), so each step's output is a single
    4-partition DMA; ctx/pos input DMAs are likewise merged via
    rearranged access patterns.
  * The cell state c is kept in bf16 (tolerance is 2e-2) so every
    c/h elementwise op runs in the DVE's 2x packed mode.

Device output layout is [12, 2, NC] per core (batch-contiguous);
the host glues the 8 shards and transposes to [N, 12, 2].
"""

import numpy as np

import concourse.bass as bass
import concourse.bacc as bacc_mod
import concourse.mybir as mybir
import concourse.tile as tile
from concourse.bass import ds, ts
from concourse.bass_utils import run_bass_kernel_spmd

N_CORES = 8
N_TOTAL = 262144
NCB = N_TOTAL // N_CORES  # 32768 batch rows per core
H = 64
PRED = 12
NB = 512                 # batch elements per chunk (one PSUM bank @ fp32)
PAIRS = NCB // (2 * NB)  # 32 chunk-pairs per core
NVAR = 8                 # pos/delta partition-row variants
INTERLEAVE = 4           # pairs software-pipelined together

F32 = mybir.dt.float32
BF16 = mybir.dt.bfloat16
AF = mybir.ActivationFunctionType
ALU = mybir.AluOpType

# gates tile bank order: I, F, O (sigmoid, one merged ACT op) ; G separate
_BANK_GATE = ("i", "o", "f", "g")
_GATE_SLICE = {"i": slice(0, 64), "f": slice(64, 128),
               "g": slice(128, 192), "o": slice(192, 256)}

# weight-block layout (128 cols each) in the packed [128, NWB*128] tensor:
#   0..3           W_HH   (I, F, O, G)
#   4..7           W_CTX  (I, F, O, G)
#   8..8+4*NVAR-1  W_POS  variant-major: v*4 + bank
#   8+4*NVAR..+NVAR-1   W_FC variants
#   last           FCBS: col v holds fc_b at the variant's 4 rows
_POS0 = 8
_FC0 = _POS0 + 4 * NVAR
_FCB0 = _FC0 + NVAR
N_WBLK = _FCB0 + 1


def _var_rows(v):
    # X pos at rows {4v, 4v+1}, Y pos adjacent at {4v+2, 4v+3}: the output
    # (and input) pos DMAs then move 4 contiguous partitions in one shot
    return 4 * v, 4 * v + 2


def build_weight_blocks(W_ih, W_hh, b_ih, b_hh, fc_w, fc_b):
    b = (b_ih + b_hh).astype(np.float32)
    out = np.zeros((128, N_WBLK * 128), dtype=np.float32)

    def blk(j):
        return out[:, 128 * j:128 * (j + 1)]

    fcw_t = fc_w.T.astype(np.float32)                     # [64, 2]
    for j, gate in enumerate(_BANK_GATE):
        sl = _GATE_SLICE[gate]
        wp_t = W_ih[sl, 0:2].T.astype(np.float32)         # [2, 64(out)]
        # pos-chain decoupling: substitute pos_t = pos_{t-1} + h_t@fcw.T+fcb
        # into the gate equation, so gates at step t depend on pos_{t-1}
        # (one extra step of pipeline slack).  h carries the correction via
        # Whh' = Whh.T + fcw.T @ Wp.T and the bias row gains fcb @ Wp.T;
        # step 0 is fixed up by streaming (pos_0 - fcb) instead of pos_0.
        whh_t = W_hh[sl, :].T.astype(np.float32) + fcw_t @ wp_t
        wc_t = W_ih[sl, 2:66].T.astype(np.float32)        # [64(ctx), 64(out)]
        bg = b[sl] + fc_b.astype(np.float32) @ wp_t

        blk(j)[0:64, 0:64] = whh_t
        blk(j)[64:128, 64:128] = whh_t
        blk(4 + j)[0:64, 0:64] = wc_t
        blk(4 + j)[64:128, 64:128] = wc_t

        for v in range(NVAR):
            rx, ry = _var_rows(v)
            pb = blk(_POS0 + 4 * v + j)
            pb[rx:rx + 2, 0:64] = wp_t
            pb[32, 0:64] = bg
            pb[ry:ry + 2, 64:128] = wp_t
            pb[96, 64:128] = bg

    fcbv = np.zeros((128, NVAR), dtype=np.float32)
    for v in range(NVAR):
        rx, ry = _var_rows(v)
        fb = blk(_FC0 + v)
        fb[0:64, rx:rx + 2] = fcw_t
        fb[64:128, ry:ry + 2] = fcw_t
        fcbv[rx:rx + 2, v] = fc_b.astype(np.float32)
        fcbv[ry:ry + 2, v] = fc_b.astype(np.float32)
    return out, fcbv


class _PairState:
    __slots__ = ("p", "v", "xb", "yb", "ctx", "pos", "posb", "h", "c",
                 "posb_nxt", "ifo", "g", "p1", "p2", "tc", "delta")


def build_bass(pairs=PAIRS, repeat=1):
    """Trace the per-core Tile kernel (identical on all 8 cores)."""
    nc = bacc_mod.Bacc()
    ctxT = nc.declare_dram_parameter("ctxT", [H, NCB], BF16, isOutput=False)
    posT = nc.declare_dram_parameter("posT", [2, NCB], F32, isOutput=False)
    wblk = nc.declare_dram_parameter("wblk", [128, N_WBLK * 128], BF16,
                                     isOutput=False)
    fcbv = nc.declare_dram_parameter("fcbv", [128, NVAR], F32, isOutput=False)
    out = nc.declare_dram_parameter("out", [PRED, 2, NCB], F32, isOutput=True)

    with tile.TileContext(nc) as tc:
        with (
            tc.tile_pool(name="wpool", bufs=1) as wpool,
            tc.tile_pool(name="sb", bufs=2) as sb,
            tc.tile_pool(name="psum", bufs=2, space="PSUM") as psum,
        ):
            wt = wpool.tile([128, N_WBLK * 128], BF16, name="wt")
            nc.sync.dma_start(out=wt[:, :], in_=wblk[:, :])
            fcbt = wpool.tile([128, NVAR], F32, name="fcbt")
            nc.sync.dma_start(out=fcbt[:, :], in_=fcbv[:, :])
            WHH = [wt[:, ts(j, 128)] for j in range(4)]
            WCTX = [wt[:, ts(4 + j, 128)] for j in range(4)]

            def wpos(v, bk):
                return wt[:, ts(_POS0 + 4 * v + bk, 128)]

            def wfc(v):
                return wt[:, ts(_FC0 + v, 128)]

            def fcb(v):
                return fcbt[:, v:v + 1]

            def prologue(st):
                j = st.p % INTERLEAVE
                st.ctx = sb.tile([128, NB], BF16, tag=f"ctx{j}")
                nc.sync.dma_start(
                    out=st.ctx[:, :],
                    in_=ctxT[:, ds(st.xb, 2 * NB)].rearrange(
                        "h (c b) -> c h b", c=2))
                rx, ry = _var_rows(st.v)
                st.pos = sb.tile([128, NB], F32, tag=f"pos{j}", bufs=3)
                nc.vector.memset(st.pos[:, :], 0.0)
                nc.vector.memset(st.pos[32:33, :], 1.0)
                nc.vector.memset(st.pos[96:97, :], 1.0)
                nc.sync.dma_start(
                    out=st.pos[ds(rx, 4), :],
                    in_=posT[:, ds(st.xb, 2 * NB)].rearrange(
                        "p (c b) -> c p b", c=2))
                # step 0 streams (pos_0 - fcb) through the cf-adjusted bias
                # row, which cancels back to pos_0 @ Wp.T + b exactly
                st.posb = sb.tile([128, NB], BF16, tag=f"posb{j}")
                nc.vector.tensor_scalar_sub(st.posb[:, :], st.pos[:, :],
                                            fcb(st.v))
                st.posb_nxt = sb.tile([128, NB], BF16, tag=f"posb{j}")
                nc.vector.tensor_copy(st.posb_nxt[:, :], st.pos[:, :])
                st.h = None
                st.c = None

            def emit_gates(st, t):
                """12 gate matmuls + the sigmoid/tanh reads of the banks."""
                j = st.p % INTERLEAVE
                g1 = psum.tile([128, 3 * NB], F32, tag="g1")
                g2 = psum.tile([128, NB], F32, tag="g2")
                for bk in range(4):
                    if t == 0 and bk == 2:
                        continue  # f gate multiplies c0 == 0
                    bank = g1[:, ts(bk, NB)] if bk < 3 else g2[:, :]
                    nc.tensor.matmul(bank, WCTX[bk], st.ctx[:, :],
                                     start=True, stop=False)
                    nc.tensor.matmul(bank, wpos(st.v, bk), st.posb[:, :],
                                     start=False, stop=(t == 0))
                    if t > 0:
                        nc.tensor.matmul(bank, WHH[bk], st.h[:, :],
                                         start=False, stop=True)
                st.posb = st.posb_nxt
                st.ifo = sb.tile([128, 3 * NB], BF16, tag=f"ifo{j}")
                st.g = sb.tile([128, NB], BF16, tag=f"g{j}")
                nio = 2 * NB if t == 0 else 3 * NB
                nc.scalar.activation(st.ifo[:, 0:nio], g1[:, 0:nio],
                                     AF.Sigmoid)
                nc.scalar.activation(st.g[:, :], g2[:, :], AF.Tanh)

            def emit_cupd(st, t):
                """c_new = f*c + i*g on DVE (bf16 2x mode)."""
                j = st.p % INTERLEAVE
                c_new = sb.tile([128, NB], BF16, tag=f"c{j}")
                if t == 0:
                    nc.vector.tensor_mul(c_new[:, :], st.ifo[:, 0:NB],
                                         st.g[:, :])
                else:
                    st.p1 = sb.tile([128, NB], BF16, tag=f"p1{j}", bufs=1)
                    st.p2 = sb.tile([128, NB], BF16, tag=f"p2{j}", bufs=1)
                    nc.vector.tensor_mul(st.p1[:, :], st.ifo[:, 0:NB],
                                         st.g[:, :])
                    nc.vector.tensor_mul(st.p2[:, :], st.ifo[:, ds(2 * NB, NB)],
                                         st.c[:, :])
                    nc.vector.tensor_add(c_new[:, :], st.p1[:, :],
                                         st.p2[:, :])
                st.c = c_new

            def emit_tanh_c(st, t):
                j = st.p % INTERLEAVE
                st.tc = sb.tile([128, NB], BF16, tag=f"tc{j}", bufs=1)
                nc.scalar.activation(st.tc[:, :], st.c[:, :], AF.Tanh)

            def emit_h(st, t):
                j = st.p % INTERLEAVE
                h_new = sb.tile([128, NB], BF16, tag=f"h{j}")
                nc.vector.tensor_mul(h_new[:, :], st.ifo[:, ds(NB, NB)],
                                     st.tc[:, :])
                st.h = h_new

            def emit_fc(st, t):
                st.delta = psum.tile([128, NB], F32, tag="g2")
                nc.tensor.matmul(st.delta[:, :], wfc(st.v), st.h[:, :],
                                 start=True, stop=True)

            def emit_pos(st, t):
                j = st.p % INTERLEAVE
                rx, ry = _var_rows(st.v)
                pos_new = sb.tile([128, NB], F32, tag=f"pos{j}", bufs=3)
                nc.vector.scalar_tensor_tensor(
                    out=pos_new[:, :], in0=st.delta[:, :], scalar=fcb(st.v),
                    in1=st.pos[:, :], op0=ALU.add, op1=ALU.add)
                if t + 2 < PRED:
                    posb_new = sb.tile([128, NB], BF16, tag=f"posb{j}")
                    nc.vector.tensor_copy(posb_new[:, :], pos_new[:, :])
                    st.posb_nxt = posb_new
                nc.sync.dma_start(
                    out=out[t, :, ds(st.xb, 2 * NB)].rearrange(
                        "p (c b) -> c p b", c=2),
                    in_=pos_new[ds(rx, 4), :])
                st.pos = pos_new

            def make_group(done, grp):
                sts = []
                for k in range(grp):
                    st = _PairState()
                    st.p = done + k
                    st.v = st.p % NVAR
                    st.xb = (2 * st.p) * NB
                    st.yb = (2 * st.p + 1) * NB
                    prologue(st)
                    sts.append(st)
                return sts

            for rep in range(repeat):
                done = 0
                tail = []
                sts = make_group(0, min(INTERLEAVE, pairs))
                while sts:
                    grp = len(sts)
                    done += grp
                    nxt = None
                    for t in range(PRED):
                        if t == 0 and tail:
                            # previous group's final fc/pos fills the PE dip
                            # during this group's light t=0 step
                            for st in tail:
                                emit_fc(st, PRED - 1)
                                emit_pos(st, PRED - 1)
                            tail = []
                        if t > 0:
                            # delta shares the g2 PSUM tag (bufs=2): fuse
                            # fc+pos per pair so each delta's stt read is
                            # emitted before the 3rd pair recycles its buffer
                            for st in sts:
                                emit_fc(st, t - 1)
                                emit_pos(st, t - 1)
                        if t == PRED - 1 and done < pairs:
                            # rolling prologue: next group's input DMAs and
                            # memsets overlap this group's tail latency chain
                            nxt = make_group(done, min(INTERLEAVE,
                                                       pairs - done))
                        for st in sts:
                            # inline per pair: tanh(c) lands between this
                            # pair's tg and the next pair's sigmoid, filling
                            # the ACT engine's wait-for-matmul gaps
                            emit_gates(st, t)
                            emit_cupd(st, t)
                            emit_tanh_c(st, t)
                            emit_h(st, t)
                    if nxt is not None:
                        tail = sts
                        sts = nxt
                    else:
                        for st in sts:
                            emit_fc(st, PRED - 1)
                            emit_pos(st, PRED - 1)
                        sts = []
    nc.finalize()
    return nc


def make_in_maps(context, start_pos, W_ih, W_hh, b_ih, b_hh, fc_w, fc_b):
    context = np.asarray(context, dtype=np.float32)
    start_pos = np.asarray(start_pos, dtype=np.float32)
    import ml_dtypes
    ctxT = np.ascontiguousarray(context.T.astype(ml_dtypes.bfloat16))  # [64, N]
    posT = np.ascontiguousarray(start_pos.T)    # [2, N]
    wb, fcbv = build_weight_blocks(
        np.asarray(W_ih, np.float32), np.asarray(W_hh, np.float32),
        np.asarray(b_ih, np.float32), np.asarray(b_hh, np.float32),
        np.asarray(fc_w, np.float32), np.asarray(fc_b, np.float32))
    wb = wb.astype(ml_dtypes.bfloat16)
    in_maps = []
    for c in range(N_CORES):
        sl = slice(c * NCB, (c + 1) * NCB)
        in_maps.append({
            "ctxT": np.ascontiguousarray(ctxT[:, sl]),
            "posT": np.ascontiguousarray(posT[:, sl]),
            "wblk": wb,
            "fcbv": fcbv,
        })
    return in_maps


def gather_output(results):
    outs = [np.asarray(results[c]["out"]) for c in range(N_CORES)]
    full = np.concatenate(outs, axis=2)          # [12, 2, N]
    return np.ascontiguousarray(np.transpose(full, (2, 0, 1)).astype(np.float32))


def kernel(context, start_pos, W_ih, W_hh, b_ih, b_hh, fc_w, fc_b):
    in_maps = make_in_maps(context, start_pos, W_ih, W_hh, b_ih, b_hh,
                           fc_w, fc_b)
    nc = build_bass()
    res = run_bass_kernel_spmd(nc, in_maps, list(range(N_CORES)))
    return gather_output(res.results)


# revision 30
# speedup vs baseline: 1824.9840x; 1.4606x over previous
"""Trainium2 Bass kernel for nn_DecoderLSTM (N=262144, H=64, IN=66, PRED_LEN=12).

Strategy (pure data parallel over 8 NeuronCores, N/8 = 32768 rows per core):

  * All state is kept TRANSPOSED on chip: [feature-dim on partitions,
    batch on free dim].  The recurrent matmuls then need no per-step
    transposes: gates_T[gate, b] = W @ x_T, computed as PE matmuls with
    the small weights as the stationary operand (lhsT) and the batch
    streaming (rhs).
  * Batch is processed in pairs of 512-element chunks: chunk X lives on
    partitions 0..63, chunk Y on partitions 64..127 of every tile, so
    every DVE/ACT instruction runs at the full 128-partition width.
    The per-gate weights are embedded as 128x128 block-diagonal (or
    block-structured) matrices so one matmul produces one gate for both
    chunks.
  * Four pairs are software-pipelined.  The pos feedback is decoupled
    from the critical path by folding the fc head into the recurrent
    weights (Whh' = Whh.T + fcw.T @ Wp.T, bias += fcb @ Wp.T, and step 0
    streams pos_0 - fcb), so gates at step t need only pos_{t-1}: the
    fc matmul / pos update / bf16 copy all run one step behind with a
    full step of slack, and no engine's in-order queue blocks while
    another pair has ready work.  tanh(c) is emitted inline per pair so
    it fills the ACT engine's wait-for-matmul gaps.
  * PSUM: one [128,1536] I|O|F tile (bufs=2) + one [128,512] tile shared
    by G and the fc delta (bufs=2) = exactly 8 banks.  Bank order I|O|F
    lets step 0 skip the F gate entirely (it multiplies c_0 == 0).
  * The constant-over-time context contribution is re-streamed into the
    PSUM accumulation each step.  The position (2 dims) + gate bias ride
    a K=3 matmul whose stationary block contains [Wp.T; bias-row]; rows
    32/96 of the pos tile are pinned to 1.0 so the bias comes for free.
  * pos_{t+1} = pos_t + (h_{t+1} @ fc_w.T) + fc_b is one
    scalar_tensor_tensor on DVE; each step's positions are DMAd straight
    to DRAM.  X/Y chunk pos rows are adjacent ({4v..4v+3}, rotating over
    8 variants to spread DMA queues), so each step's output is a single
    4-partition DMA; ctx/pos input DMAs are likewise merged via
    rearranged access patterns.
  * The cell state c is kept in bf16 (tolerance is 2e-2) so every
    c/h elementwise op runs in the DVE's 2x packed mode.

Device output layout is [12, 2, NC] per core (batch-contiguous);
the host glues the 8 shards and transposes to [N, 12, 2].
"""

import numpy as np

import concourse.bass as bass
import concourse.bacc as bacc_mod
import concourse.mybir as mybir
import concourse.tile as tile
from concourse.bass import ds, ts
from concourse.bass_utils import run_bass_kernel_spmd

N_CORES = 8
N_TOTAL = 262144
NCB = N_TOTAL // N_CORES  # 32768 batch rows per core
H = 64
PRED = 12
NB = 512                 # batch elements per chunk (one PSUM bank @ fp32)
PAIRS = NCB // (2 * NB)  # 32 chunk-pairs per core
NVAR = 8                 # pos/delta partition-row variants
INTERLEAVE = 4           # pairs software-pipelined together

F32 = mybir.dt.float32
BF16 = mybir.dt.bfloat16
AF = mybir.ActivationFunctionType
ALU = mybir.AluOpType

# gates tile bank order: I, F, O (sigmoid, one merged ACT op) ; G separate
_BANK_GATE = ("i", "o", "f", "g")
_GATE_SLICE = {"i": slice(0, 64), "f": slice(64, 128),
               "g": slice(128, 192), "o": slice(192, 256)}

# weight-block layout (128 cols each) in the packed [128, NWB*128] tensor:
#   0..3           W_HH   (I, F, O, G)
#   4..7           W_CTX  (I, F, O, G)
#   8..8+4*NVAR-1  W_POS  variant-major: v*4 + bank
#   8+4*NVAR..+NVAR-1   W_FC variants
#   last           FCBS: col v holds fc_b at the variant's 4 rows
_POS0 = 8
_FC0 = _POS0 + 4 * NVAR
_FCB0 = _FC0 + NVAR
N_WBLK = _FCB0 + 1


def _var_rows(v):
    # X pos at rows {4v, 4v+1}, Y pos adjacent at {4v+2, 4v+3}: the output
    # (and input) pos DMAs then move 4 contiguous partitions in one shot
    return 4 * v, 4 * v + 2


def build_weight_blocks(W_ih, W_hh, b_ih, b_hh, fc_w, fc_b):
    b = (b_ih + b_hh).astype(np.float32)
    out = np.zeros((128, N_WBLK * 128), dtype=np.float32)

    def blk(j):
        return out[:, 128 * j:128 * (j + 1)]

    fcw_t = fc_w.T.astype(np.float32)                     # [64, 2]
    for j, gate in enumerate(_BANK_GATE):
        sl = _GATE_SLICE[gate]
        wp_t = W_ih[sl, 0:2].T.astype(np.float32)         # [2, 64(out)]
        # pos-chain decoupling: substitute pos_t = pos_{t-1} + h_t@fcw.T+fcb
        # into the gate equation, so gates at step t depend on pos_{t-1}
        # (one extra step of pipeline slack).  h carries the correction via
        # Whh' = Whh.T + fcw.T @ Wp.T and the bias row gains fcb @ Wp.T;
        # step 0 is fixed up by streaming (pos_0 - fcb) instead of pos_0.
        whh_t = W_hh[sl, :].T.astype(np.float32) + fcw_t @ wp_t
        wc_t = W_ih[sl, 2:66].T.astype(np.float32)        # [64(ctx), 64(out)]
        bg = b[sl] + fc_b.astype(np.float32) @ wp_t

        blk(j)[0:64, 0:64] = whh_t
        blk(j)[64:128, 64:128] = whh_t
        blk(4 + j)[0:64, 0:64] = wc_t
        blk(4 + j)[64:128, 64:128] = wc_t

        for v in range(NVAR):
            rx, ry = _var_rows(v)
            pb = blk(_POS0 + 4 * v + j)
            pb[rx:rx + 2, 0:64] = wp_t
            pb[32, 0:64] = bg
            pb[ry:ry + 2, 64:128] = wp_t
            pb[96, 64:128] = bg

    fcbv = np.zeros((128, NVAR), dtype=np.float32)
    for v in range(NVAR):
        rx, ry = _var_rows(v)
        fb = blk(_FC0 + v)
        fb[0:64, rx:rx + 2] = fcw_t
        fb[64:128, ry:ry + 2] = fcw_t
        fcbv[rx:rx + 2, v] = fc_b.astype(np.float32)
        fcbv[ry:ry + 2, v] = fc_b.astype(np.float32)
    return out, fcbv


class _PairState:
    __slots__ = ("p", "v", "xb", "yb", "ctx", "pos", "posb", "h", "c",
                 "posb_nxt", "ifo", "g", "p1", "p2", "tc", "delta")


def build_bass(pairs=PAIRS, repeat=1):
    """Trace the per-core Tile kernel (identical on all 8 cores)."""
    nc = bacc_mod.Bacc()
    ctxT = nc.declare_dram_parameter("ctxT", [H, NCB], BF16, isOutput=False)
    posT = nc.declare_dram_parameter("posT", [2, NCB], F32, isOutput=False)
    wblk = nc.declare_dram_parameter("wblk", [128, N_WBLK * 128], BF16,
                                     isOutput=False)
    fcbv = nc.declare_dram_parameter("fcbv", [128, NVAR], F32, isOutput=False)
    out = nc.declare_dram_parameter("out", [PRED, 2, NCB], F32, isOutput=True)

    with tile.TileContext(nc) as tc:
        with (
            tc.tile_pool(name="wpool", bufs=1) as wpool,
            tc.tile_pool(name="sb", bufs=2) as sb,
            tc.tile_pool(name="psum", bufs=2, space="PSUM") as psum,
        ):
            wt = wpool.tile([128, N_WBLK * 128], BF16, name="wt")
            nc.sync.dma_start(out=wt[:, :], in_=wblk[:, :])
            fcbt = wpool.tile([128, NVAR], F32, name="fcbt")
            nc.sync.dma_start(out=fcbt[:, :], in_=fcbv[:, :])
            WHH = [wt[:, ts(j, 128)] for j in range(4)]
            WCTX = [wt[:, ts(4 + j, 128)] for j in range(4)]

            def wpos(v, bk):
                return wt[:, ts(_POS0 + 4 * v + bk, 128)]

            def wfc(v):
                return wt[:, ts(_FC0 + v, 128)]

            def fcb(v):
                return fcbt[:, v:v + 1]

            def prologue(st):
                j = st.p % INTERLEAVE
                st.ctx = sb.tile([128, NB], BF16, tag=f"ctx{j}")
                nc.sync.dma_start(
                    out=st.ctx[:, :],
                    in_=ctxT[:, ds(st.xb, 2 * NB)].rearrange(
                        "h (c b) -> c h b", c=2))
                rx, ry = _var_rows(st.v)
                st.pos = sb.tile([128, NB], F32, tag=f"pos{j}", bufs=3)
                nc.vector.memset(st.pos[:, :], 0.0)
                nc.vector.memset(st.pos[32:33, :], 1.0)
                nc.vector.memset(st.pos[96:97, :], 1.0)
                nc.sync.dma_start(
                    out=st.pos[ds(rx, 4), :],
                    in_=posT[:, ds(st.xb, 2 * NB)].rearrange(
                        "p (c b) -> c p b", c=2))
                # step 0 streams (pos_0 - fcb) through the cf-adjusted bias
                # row, which cancels back to pos_0 @ Wp.T + b exactly
                st.posb = sb.tile([128, NB], BF16, tag=f"posb{j}")
                nc.vector.tensor_scalar_sub(st.posb[:, :], st.pos[:, :],
                                            fcb(st.v))
                st.posb_nxt = sb.tile([128, NB], BF16, tag=f"posb{j}")
                nc.vector.tensor_copy(st.posb_nxt[:, :], st.pos[:, :])
                st.h = None
                st.c = None

            def emit_gates(st, t):
                """12 gate matmuls + the sigmoid/tanh reads of the banks."""
                j = st.p % INTERLEAVE
                g1 = psum.tile([128, 3 * NB], F32, tag="g1")
                g2 = psum.tile([128, NB], F32, tag="g2")
                for bk in range(4):
                    if t == 0 and bk == 2:
                        continue  # f gate multiplies c0 == 0
                    bank = g1[:, ts(bk, NB)] if bk < 3 else g2[:, :]
                    nc.tensor.matmul(bank, WCTX[bk], st.ctx[:, :],
                                     start=True, stop=False)
                    nc.tensor.matmul(bank, wpos(st.v, bk), st.posb[:, :],
                                     start=False, stop=(t == 0))
                    if t > 0:
                        nc.tensor.matmul(bank, WHH[bk], st.h[:, :],
                                         start=False, stop=True)
                st.posb = st.posb_nxt
                st.ifo = sb.tile([128, 3 * NB], BF16, tag=f"ifo{j}")
                st.g = sb.tile([128, NB], BF16, tag=f"g{j}")
                nio = 2 * NB if t == 0 else 3 * NB
                nc.scalar.activation(st.ifo[:, 0:nio], g1[:, 0:nio],
                                     AF.Sigmoid)
                nc.scalar.activation(st.g[:, :], g2[:, :], AF.Tanh)

            def emit_cupd(st, t):
                """c_new = f*c + i*g on DVE (bf16 2x mode)."""
                j = st.p % INTERLEAVE
                c_new = sb.tile([128, NB], BF16, tag=f"c{j}")
                if t == 0:
                    nc.vector.tensor_mul(c_new[:, :], st.ifo[:, 0:NB],
                                         st.g[:, :])
                else:
                    st.p1 = sb.tile([128, NB], BF16, tag=f"p1{j}", bufs=1)
                    st.p2 = sb.tile([128, NB], BF16, tag=f"p2{j}", bufs=1)
                    nc.vector.tensor_mul(st.p1[:, :], st.ifo[:, 0:NB],
                                         st.g[:, :])
                    nc.vector.tensor_mul(st.p2[:, :], st.ifo[:, ds(2 * NB, NB)],
                                         st.c[:, :])
                    nc.vector.tensor_add(c_new[:, :], st.p1[:, :],
                                         st.p2[:, :])
                st.c = c_new

            def emit_tanh_c(st, t):
                j = st.p % INTERLEAVE
                st.tc = sb.tile([128, NB], BF16, tag=f"tc{j}", bufs=1)
                nc.scalar.activation(st.tc[:, :], st.c[:, :], AF.Tanh)

            def emit_h(st, t):
                j = st.p % INTERLEAVE
                h_new = sb.tile([128, NB], BF16, tag=f"h{j}")
                nc.vector.tensor_mul(h_new[:, :], st.ifo[:, ds(NB, NB)],
                                     st.tc[:, :])
                st.h = h_new

            def emit_fc(st, t):
                st.delta = psum.tile([128, NB], F32, tag="g2")
                nc.tensor.matmul(st.delta[:, :], wfc(st.v), st.h[:, :],
                                 start=True, stop=True)

            def emit_pos(st, t):
                j = st.p % INTERLEAVE
                rx, ry = _var_rows(st.v)
                pos_new = sb.tile([128, NB], F32, tag=f"pos{j}", bufs=3)
                nc.vector.scalar_tensor_tensor(
                    out=pos_new[:, :], in0=st.delta[:, :], scalar=fcb(st.v),
                    in1=st.pos[:, :], op0=ALU.add, op1=ALU.add)
                if t + 2 < PRED:
                    posb_new = sb.tile([128, NB], BF16, tag=f"posb{j}")
                    nc.vector.tensor_copy(posb_new[:, :], pos_new[:, :])
                    st.posb_nxt = posb_new
                nc.sync.dma_start(
                    out=out[t, :, ds(st.xb, 2 * NB)].rearrange(
                        "p (c b) -> c p b", c=2),
                    in_=pos_new[ds(rx, 4), :])
                st.pos = pos_new

            def make_group(done, grp):
                sts = []
                for k in range(grp):
                    st = _PairState()
                    st.p = done + k
                    st.v = st.p % NVAR
                    st.xb = (2 * st.p) * NB
                    st.yb = (2 * st.p + 1) * NB
                    prologue(st)
                    sts.append(st)
                return sts

            for rep in range(repeat):
                done = 0
                tail = []
                warm = False
                sts = make_group(0, min(INTERLEAVE, pairs))
                while sts:
                    grp = len(sts)
                    done += grp
                    nxt = None
                    start_t = 1 if warm else 0
                    for t in range(start_t, PRED):
                        if t == start_t and tail:
                            # previous group's final fc/pos fills the PE dip
                            # during this group's light t=0 step
                            for st in tail:
                                emit_fc(st, PRED - 1)
                                emit_pos(st, PRED - 1)
                            tail = []
                        if t > 0:
                            # delta shares the g2 PSUM tag (bufs=2): fuse
                            # fc+pos per pair so each delta's stt read is
                            # emitted before the 3rd pair recycles its buffer
                            for st in sts:
                                emit_fc(st, t - 1)
                                emit_pos(st, t - 1)
                        if t == PRED - 1 and done < pairs:
                            # rolling prologue: next group's input DMAs and
                            # memsets overlap this group's tail latency chain
                            nxt = make_group(done, min(INTERLEAVE,
                                                       pairs - done))
                        for st in sts:
                            # inline per pair: tanh(c) lands between this
                            # pair's tg and the next pair's sigmoid, filling
                            # the ACT engine's wait-for-matmul gaps
                            emit_gates(st, t)
                            emit_cupd(st, t)
                            emit_tanh_c(st, t)
                            emit_h(st, t)
                        if t == PRED - 1 and nxt is not None:
                            # pre-start the next group's step 0 (it depends
                            # only on its own prologue): its gate chain
                            # overlaps this group's tail, killing the
                            # pipeline-refill dip at the group swap
                            for st in nxt:
                                emit_gates(st, 0)
                                emit_cupd(st, 0)
                                emit_tanh_c(st, 0)
                                emit_h(st, 0)
                    if nxt is not None:
                        tail = sts
                        sts = nxt
                        warm = True
                    else:
                        for st in sts:
                            emit_fc(st, PRED - 1)
                            emit_pos(st, PRED - 1)
                        sts = []
    nc.finalize()
    return nc


def make_in_maps(context, start_pos, W_ih, W_hh, b_ih, b_hh, fc_w, fc_b):
    context = np.asarray(context, dtype=np.float32)
    start_pos = np.asarray(start_pos, dtype=np.float32)
    import ml_dtypes
    ctxT = np.ascontiguousarray(context.T.astype(ml_dtypes.bfloat16))  # [64, N]
    posT = np.ascontiguousarray(start_pos.T)    # [2, N]
    wb, fcbv = build_weight_blocks(
        np.asarray(W_ih, np.float32), np.asarray(W_hh, np.float32),
        np.asarray(b_ih, np.float32), np.asarray(b_hh, np.float32),
        np.asarray(fc_w, np.float32), np.asarray(fc_b, np.float32))
    wb = wb.astype(ml_dtypes.bfloat16)
    in_maps = []
    for c in range(N_CORES):
        sl = slice(c * NCB, (c + 1) * NCB)
        in_maps.append({
            "ctxT": np.ascontiguousarray(ctxT[:, sl]),
            "posT": np.ascontiguousarray(posT[:, sl]),
            "wblk": wb,
            "fcbv": fcbv,
        })
    return in_maps


def gather_output(results):
    outs = [np.asarray(results[c]["out"]) for c in range(N_CORES)]
    full = np.concatenate(outs, axis=2)          # [12, 2, N]
    return np.ascontiguousarray(np.transpose(full, (2, 0, 1)).astype(np.float32))


def kernel(context, start_pos, W_ih, W_hh, b_ih, b_hh, fc_w, fc_b):
    in_maps = make_in_maps(context, start_pos, W_ih, W_hh, b_ih, b_hh,
                           fc_w, fc_b)
    nc = build_bass()
    res = run_bass_kernel_spmd(nc, in_maps, list(range(N_CORES)))
    return gather_output(res.results)
